# revision 1
# baseline (speedup 1.0000x reference)
"""Top-1 MoE routing layer (HCE Linear) on 8 Trainium2 NeuronCores.

y[b] = x[b] @ W[argmax_e sigmoid(x @ Wp.T + bp)[b, e]]   (multi-hot on exact ties)

Strategy: data-parallel over tokens. The router (a [B,8] matmul + argmax —
~0.03% of the FLOPs) is computed on host in fp32 with exactly the reference
semantics; tokens are then grouped by expert into capacity-padded segments so
all 8 cores run one identical (SPMD) Bass program that is just dense
per-segment fp32 matmuls at the HBM roofline (~3 MB/core: 2 MB weights +
~0.5 MB activations each way).
"""

from contextlib import ExitStack

import numpy as np

import bass_rust
import concourse.bass as bass
import concourse.tile as tile
from concourse import mybir
from concourse.bass_utils import run_bass_kernel_spmd
from concourse.vector_clock import ScopedClock

NCORES = 8


class _SplitDrainTileContext(tile.TileContext):
    """TileContext legalized for a walrus build that allows at most ONE sem
    wait per instruction ("Too many sync wait commands" otherwise).

    Extra waits are hoisted onto same-engine InstNoOp carriers placed
    immediately before the owning instruction (identical semantics: the
    engine sequencer executes them in order), and the kernel-tail drain is
    split into a chain of single-wait drains.
    """

    _wait_nop_counter = 0

    def _lower_ordered_insts(self, ordered):
        for bb_name, insts in list(ordered.items()):
            out = []
            for inst in insts:
                si = getattr(inst, "sync_info", None)
                waits = list(si.on_wait) if si is not None else []
                if len(waits) > 1:
                    for w in waits[:-1]:
                        type(self)._wait_nop_counter += 1
                        nop = mybir.InstNoOp(
                            name=f"waitnop_{type(self)._wait_nop_counter}",
                            engine=inst.engine,
                            sync_info=mybir.SyncInfo(on_wait=[w], on_update=[]),
                            bass_nofuse=True,
                        )
                        out.append(nop)
                    inst.sync_info = mybir.SyncInfo(
                        on_wait=[waits[-1]], on_update=list(si.on_update)
                    )
                out.append(inst)
            ordered[bb_name] = out
        return super()._lower_ordered_insts(ordered)

    def _drain_and_barrier(self, tick_clock, wait_clock):
        drain_inst = self.nc.sync.drain()
        wait_clock.add_sem_waits(
            drain_inst.ins, ScopedClock({None: tick_clock.global_clock})
        )
        si = drain_inst.ins.sync_info
        waits = list(si.on_wait)
        if len(waits) > 1:
            # strip the drain; carry each wait on a cheap nop instead of a
            # chain of full drains (those cost ~100ns each)
            drain_inst.ins.sync_info = bass_rust.SyncInfo(
                on_wait=[], on_update=list(si.on_update)
            )
            for w in waits:
                n2 = self.nc.sync.nop(nofuse=True)
                n2.ins.sync_info = bass_rust.SyncInfo(on_wait=[w], on_update=[])
        self.nc.all_engine_barrier(sem_only=True)
        assert self.sems is not None
        popped = self.nc._tile_sem_poison_stack.pop()
        assert popped is self._sem_poison
        self.nc.clear_and_free_semaphores(list(self.sems.allocated().values()))


def _build_program(I, O, E, C):
    """One SPMD core program: yT[o, seg] = W[e(seg)].T @ xT[:, seg].

    Inputs (per core):
      wk [2, 128, E*2*128] fp32 — wk[ot, p, (e*2+kt)*128 + c] = W[e, kt*128+p, ot*128+c]
      xk [128, 2, E*C]     fp32 — xk[p, kt, e*C+j] = x_token(e,j)[kt*128+p]
    Output:
      yk [2, 128, E*C]     fp32 — yk[ot, p, col] = y_col[ot*128+p]
    """
    assert I == 256 and O == 256, "packed layout assumes 256x256 experts"
    KT = I // 128  # 2
    OT = O // 128  # 2
    S = E * C  # total columns per core
    dt = mybir.dt.float32

    nc = bass.Bass("TRN2", target_bir_lowering=False, debug=False, num_devices=NCORES)
    wk = nc.dram_tensor("wk", [OT, 128, E * KT * 128], dt, kind="ExternalInput").ap()
    xk = nc.dram_tensor("xk", [128, KT, S], dt, kind="ExternalInput").ap()
    yk = nc.dram_tensor("yk", [OT, 128, S], dt, kind="ExternalOutput").ap()

    # segment -> bank-aligned matmul pieces (PSUM bank = 512 fp32 columns)
    pieces = []  # (e, col_start, n)
    for e in range(E):
        s0, s1 = e * C, (e + 1) * C
        while s0 < s1:
            nxt = min(s1, (s0 // 512 + 1) * 512)
            pieces.append((e, s0, nxt - s0))
            s0 = nxt

    with _SplitDrainTileContext(nc) as tc:
        with ExitStack() as ctx:
            wpool = ctx.enter_context(tc.tile_pool(name="w", bufs=OT))
            xpool = ctx.enter_context(tc.tile_pool(name="x", bufs=1))
            ppool = ctx.enter_context(tc.tile_pool(name="ps", bufs=OT, space="PSUM"))
            ypool = ctx.enter_context(tc.tile_pool(name="y", bufs=OT))

            # two HWDGE rings (SP + ACT). x halves land first (they gate all
            # matmuls); W arrives in tapered chunks per ot so the last chunk
            # gates as little work as possible.
            XH = S // 2
            sbx = []
            for h, eng in ((0, nc.scalar), (1, nc.sync)):
                t = xpool.tile([128, KT, XH], dt, tag=f"x{h}")
                sbx.append((t, eng))

            def rhs_ap(kt, s0, n):
                h = s0 // XH
                assert (s0 + n - 1) // XH == h, (s0, n)
                return sbx[h][0][:, kt, s0 - h * XH : s0 - h * XH + n]

            # tapered expert chunks; big chunks first on each ring, small last
            CH = [(0, 4), (4, 3), (7, 1)]  # (first expert, n experts)
            sbw = {}

            def issue_w(ot, ci, eng):
                e0, ne = CH[ci]
                t = wpool.tile([128, ne * KT * 128], dt, tag=f"w{ot}{ci}")
                lo = e0 * KT * 128
                eng.dma_start(out=t[:], in_=wk[ot, :, lo : lo + ne * KT * 128])
                sbw[(ot, ci)] = t

            def issue_x(h):
                t, eng = sbx[h]
                eng.dma_start(out=t[:], in_=xk[:, :, h * XH : (h + 1) * XH])

            issue_x(0)
            issue_x(1)
            issue_w(1, 0, nc.sync)
            issue_w(0, 0, nc.scalar)
            issue_w(0, 1, nc.sync)
            issue_w(1, 1, nc.scalar)
            issue_w(0, 2, nc.sync)
            issue_w(1, 2, nc.scalar)

            ps = []
            sby = []
            for ot in range(OT):
                ps_t = ppool.tile([128, S], dt, tag=f"ps{ot}")
                ps.append(ps_t)
                sby_t = ypool.tile([128, S], dt, tag=f"sy{ot}")
                sby.append(sby_t)
            for ci, (e0, ne) in enumerate(CH):
                ot_order = [1, 0] if ci == 0 else list(range(OT))
                for ot in ot_order:
                    for e, s0, n in pieces:
                        if not (e0 <= e < e0 + ne):
                            continue
                        for kt in range(KT):
                            te = ((e - e0) * KT + kt) * 128
                            nc.tensor.matmul(
                                out=ps[ot][:, s0 : s0 + n],
                                lhsT=sbw[(ot, ci)][:, te : te + 128],
                                rhs=rhs_ap(kt, s0, n),
                                start=(kt == 0),
                                stop=(kt == KT - 1),
                            )
                for ot in ot_order:
                    lo, hi = e0 * C, (e0 + ne) * C
                    nc.vector.tensor_copy(sby[ot][:, lo:hi], ps[ot][:, lo:hi])
                    eng = nc.scalar if ot == 0 else nc.sync
                    eng.dma_start(out=yk[ot, :, lo:hi], in_=sby[ot][:, lo:hi])

    return nc


_cache: dict = {}


def _get_program(I, O, E, C):
    key = (I, O, E, C)
    if key not in _cache:
        _cache[key] = _build_program(I, O, E, C)
    return _cache[key]


def _pack_inputs(x, W, Wp, bp):
    B, I = x.shape
    E, _, O = W.shape

    # --- host router: replicate reference fp32 semantics (incl. tie multi-hot)
    logits = x @ Wp.T + bp
    g = 1.0 / (1.0 + np.exp(-logits, dtype=np.float32))
    onehot = g == g.max(axis=1, keepdims=True)  # [B, E] bool, >=1 True per row
    tok_of_pair, exp_of_pair = np.nonzero(onehot)  # pairs sorted by token

    # per-expert pair lists, split evenly over cores into capacity-C segments
    order = np.argsort(exp_of_pair, kind="stable")
    toks_by_e = tok_of_pair[order]
    n_e = np.bincount(exp_of_pair, minlength=E)
    C = max(1, int(-(-n_e.max() // NCORES)))  # ceil(max_e n_e / NCORES)
    S = E * C

    # slot tables: for each (core, e, j<cnt) the source token
    src_tok = np.zeros((NCORES, S), dtype=np.int64)
    valid = np.zeros((NCORES, S), dtype=bool)
    off = 0
    for e in range(E):
        parts = np.array_split(toks_by_e[off : off + n_e[e]], NCORES)
        off += n_e[e]
        for c in range(NCORES):
            k = len(parts[c])
            src_tok[c, e * C : e * C + k] = parts[c]
            valid[c, e * C : e * C + k] = True

    # pack inputs
    wkk = (
        W.reshape(E, 2, 128, 2, 128).transpose(3, 2, 0, 1, 4).reshape(2, 128, E * 2 * 128)
    )
    wkk = np.ascontiguousarray(wkk)
    xT = np.ascontiguousarray(x.T.reshape(2, 128, B))  # [kt, p, b]
    in_maps = []
    for c in range(NCORES):
        xs = np.zeros((128, 2, S), dtype=np.float32)
        cols = np.nonzero(valid[c])[0]
        xs[:, :, cols] = xT.transpose(1, 0, 2)[:, :, src_tok[c, cols]]
        in_maps.append({"wk": wkk, "xk": xs})
    return in_maps, (C, S, src_tok, valid)


def kernel(x, W, Wp, bp):
    x = np.ascontiguousarray(np.asarray(x, dtype=np.float32))
    W = np.ascontiguousarray(np.asarray(W, dtype=np.float32))
    Wp = np.ascontiguousarray(np.asarray(Wp, dtype=np.float32))
    bp = np.ascontiguousarray(np.asarray(bp, dtype=np.float32))
    B, I = x.shape
    E, _, O = W.shape

    in_maps, (C, S, src_tok, valid) = _pack_inputs(x, W, Wp, bp)
    nc = _get_program(I, O, E, C)
    res = run_bass_kernel_spmd(nc, in_maps, list(range(NCORES)))

    # host unscatter: y[token] += yT column (add: handles tie multi-hot rows)
    y = np.zeros((B, O), dtype=np.float32)
    for c in range(NCORES):
        yc = res.results[c]["yk"]  # [2, 128, S]
        ycol = yc.transpose(2, 0, 1).reshape(S, O)  # [S, O]
        cols = np.nonzero(valid[c])[0]
        np.add.at(y, src_tok[c, cols], ycol[cols])
    return y



# revision 3
# speedup vs baseline: 1.5047x; 1.5047x over previous
"""Top-1 MoE routing layer (HCE Linear) on 8 Trainium2 NeuronCores — v2.

y[b] = x[b] @ W[argmax_e sigmoid(x @ Wp.T + bp)[b, e]]   (multi-hot on exact ties)

Strategy: EXPERT-parallel. The router runs on host (fp32, exact reference
semantics); core e receives only expert e's weight (fp16, 128KB) and the
tokens routed to it (padded to C = max_e n_e columns, fp16). Each core does
a dense [256,256] x [256,C] matmul in fp16 (fp32 PSUM accumulate) and DMAs
the fp32 result straight from PSUM to DRAM in chunks spread across engine
queues. Per-core DMA work is ~6 x 500ns minimum-cost transfers across 4
queues; the critical path is input-latency + PE + output-latency.
"""

from contextlib import ExitStack

import numpy as np

import bass_rust
import concourse.bass as bass
import concourse.tile as tile
from concourse import mybir
from concourse.bass_utils import run_bass_kernel_spmd
from concourse.vector_clock import ScopedClock

NCORES = 8


class _SplitDrainTileContext(tile.TileContext):
    """TileContext legalized for a walrus build that allows at most ONE sem
    wait per instruction ("Too many sync wait commands" otherwise).

    Extra waits are hoisted onto same-engine InstNoOp carriers placed
    immediately before the owning instruction (identical semantics: the
    engine sequencer executes them in order), and the kernel-tail drain is
    split into a chain of single-wait drains.
    """

    _wait_nop_counter = 0

    def _lower_ordered_insts(self, ordered):
        for bb_name, insts in list(ordered.items()):
            out = []
            for inst in insts:
                si = getattr(inst, "sync_info", None)
                waits = list(si.on_wait) if si is not None else []
                if len(waits) > 1:
                    for w in waits[:-1]:
                        type(self)._wait_nop_counter += 1
                        nop = mybir.InstNoOp(
                            name=f"waitnop_{type(self)._wait_nop_counter}",
                            engine=inst.engine,
                            sync_info=mybir.SyncInfo(on_wait=[w], on_update=[]),
                            bass_nofuse=True,
                        )
                        out.append(nop)
                    inst.sync_info = mybir.SyncInfo(
                        on_wait=[waits[-1]], on_update=list(si.on_update)
                    )
                out.append(inst)
            ordered[bb_name] = out
        return super()._lower_ordered_insts(ordered)

    def _drain_and_barrier(self, tick_clock, wait_clock):
        drain_inst = self.nc.sync.drain()
        wait_clock.add_sem_waits(
            drain_inst.ins, ScopedClock({None: tick_clock.global_clock})
        )
        si = drain_inst.ins.sync_info
        waits = list(si.on_wait)
        if len(waits) > 1:
            # strip the drain; carry each wait on a cheap nop instead of a
            # chain of full drains (those cost ~100ns each)
            drain_inst.ins.sync_info = bass_rust.SyncInfo(
                on_wait=[], on_update=list(si.on_update)
            )
            for w in waits:
                n2 = self.nc.sync.nop(nofuse=True)
                n2.ins.sync_info = bass_rust.SyncInfo(on_wait=[w], on_update=[])
        self.nc.all_engine_barrier(sem_only=True)
        assert self.sems is not None
        popped = self.nc._tile_sem_poison_stack.pop()
        assert popped is self._sem_poison
        self.nc.clear_and_free_semaphores(list(self.sems.allocated().values()))


SEQ_544 = [
    (0, 64, "D"),
    (0, 160, "A"),
    (0, 160, "D"),
    (1, 192, "A"),
    (0, 160, "D"),
    (1, 192, "A"),
    (1, 160, "D"),
]


def _build_program(I, O, C):
    """One SPMD core program: ps[ot][:, j] += W[ot,kt].T @ x[kt][:, j].

    Inputs (per core, expert = core id):
      wq [128, 512] fp16 — wq[p, (ot*2+kt)*128 + oc] = W[e, kt*128+p, ot*128+oc]
      xq [128, 2, C] fp16 — xq[p, kt, j] = x_token(j)[kt*128 + p]
    Output:
      yq [2, 128, C] fp16 — yq[ot, p, j] = y_token(j)[ot*128 + p]
    """
    assert I == 256 and O == 256, "packed layout assumes 256x256 experts"
    KT, OT = 2, 2
    dt16 = mybir.dt.float16
    dt32 = mybir.dt.float32

    nc = bass.Bass("TRN2", target_bir_lowering=False, debug=False, num_devices=NCORES)
    wq = nc.dram_tensor("wq", [128, OT * KT * 128], dt16, kind="ExternalInput").ap()
    xq = nc.dram_tensor("xq", [128, KT, C], dt16, kind="ExternalInput").ap()
    yq = nc.dram_tensor("yq", [OT, 128, C], dt16, kind="ExternalOutput").ap()

    # --- schedule ------------------------------------------------------
    # Chunks: (ot, n, copy_engine).  Each chunk owns one PSUM bank-tile and
    # is filled by 64-col matmul pieces (finer granularity helps while the
    # PE p-state is mid-speed, i.e. dispatched before sim time 3000), then
    # drained by ONE PSUM->SBUF copy (only DVE and ACT can read PSUM on
    # TRN2).  The interleaved ot0/ot1 order spreads the copy load across
    # the whole window; chunk sizes/engines are from an offline search of
    # the CoreSim cost model.  y-ot0 ships on Pool, y-ot1 on SP, so neither
    # queues behind an input DMA or a copy engine.
    A = min(C, 272)
    if C == 544:
        seq = SEQ_544
    else:
        # generic fallback: ~160-col chunks, ot0 leading, alternating engines
        def chop(total):
            out = []
            s = 0
            while s < total:
                n = min(192, total - s)
                out.append(n)
                s += n
            return out

        c0, c1 = chop(C), chop(C)
        seq = []
        k0 = k1 = 0
        for i in range(len(c0) + len(c1)):
            if k0 <= k1 and k0 < len(c0):
                seq.append((0, c0[k0], "D" if i % 2 == 0 else "A"))
                k0 += 1
            else:
                seq.append((1, c1[k1], "D" if i % 2 == 0 else "A"))
                k1 += 1

    with _SplitDrainTileContext(nc) as tc:
        with ExitStack() as ctx:
            wpool = ctx.enter_context(tc.tile_pool(name="w", bufs=1))
            xpool = ctx.enter_context(tc.tile_pool(name="x", bufs=1))
            ypool = ctx.enter_context(tc.tile_pool(name="y", bufs=OT))
            zpool = ctx.enter_context(tc.tile_pool(name="z", bufs=1))
            ppool = ctx.enter_context(tc.tile_pool(name="ps", bufs=8, space="PSUM"))

            wt = wpool.tile([128, OT * KT * 128], dt16, tag="w")
            nc.scalar.dma_start(out=wt[:], in_=wq[:, :])  # ACT queue
            # preload the ACT activation table (Copy) in the shadow of the
            # input DMAs so the first real ACT copy doesn't pay the ~1.4us
            # table load
            zt = zpool.tile([128, 2], dt32, tag="z")
            nc.vector.memset(zt[:], 0.0)
            nc.scalar.copy(zt[:, 1:2], zt[:, 0:1])
            # x split by COLUMN range (both kt halves in each piece) so every
            # matmul pair (kt0 start, kt1 stop) has its data as soon as its
            # piece arrives; [A,C) lands via Pool at ~2483, [0,A) via SP 2417.
            xt = xpool.tile([128, KT, C], dt16, tag="x")
            if C > A:
                nc.gpsimd.dma_start(out=xt[:, :, A:C], in_=xq[:, :, A:C])
            nc.sync.dma_start(out=xt[:, :, 0:A], in_=xq[:, :, 0:A])

            sy = []
            for ot in range(OT):
                sy_t = ypool.tile([128, C], dt16, tag=f"sy{ot}")
                sy.append(sy_t)

            lo = [0, 0]  # next column per ot
            for ot, n, ceng in seq:
                s0 = lo[ot]
                lo[ot] += n
                pt = ppool.tile([128, 512], dt32, tag="pt")
                # fill the bank with 64-col pieces (kt0 start + kt1 stop each)
                s = 0
                while s < n:
                    pn = min(64, n - s)
                    for kt in range(KT):
                        nc.tensor.matmul(
                            out=pt[:, s : s + pn],
                            lhsT=wt[:, (ot * KT + kt) * 128 : (ot * KT + kt + 1) * 128],
                            rhs=xt[:, kt, s0 + s : s0 + s + pn],
                            start=(kt == 0),
                            stop=(kt == KT - 1),
                        )
                    s += pn
                # one copy per chunk (casts fp32 -> fp16)
                if ceng == "D":
                    nc.vector.tensor_copy(sy[ot][:, s0 : s0 + n], pt[:, 0:n])
                else:
                    nc.scalar.copy(sy[ot][:, s0 : s0 + n], pt[:, 0:n])
            assert lo[0] == C and lo[1] == C
            nc.sync.dma_start(out=yq[0, :, :], in_=sy[0][:, :])  # SP
            nc.scalar.dma_start(out=yq[1, :, :], in_=sy[1][:, :])  # ACT

    return nc


_cache: dict = {}


def _get_program(I, O, C):
    key = (I, O, C)
    if key not in _cache:
        _cache[key] = _build_program(I, O, C)
    return _cache[key]


def _pack_inputs(x, W, Wp, bp):
    B, I = x.shape
    E, _, O = W.shape

    # --- host router: replicate reference fp32 semantics (incl. tie multi-hot)
    logits = x @ Wp.T + bp
    g = 1.0 / (1.0 + np.exp(-logits, dtype=np.float32))
    onehot = g == g.max(axis=1, keepdims=True)  # [B, E] bool, >=1 True per row

    toks = [np.nonzero(onehot[:, e])[0] for e in range(E)]
    C = max(1, max(len(t) for t in toks))
    # pad C so y-group/piece schedule stays sane (and >= 16 cols)
    C = max(C, 16)

    xT = np.ascontiguousarray(x.T).reshape(2, 128, B).astype(np.float16)
    wk = (
        W.reshape(E, 2, 128, 2, 128)
        .transpose(0, 2, 3, 1, 4)
        .reshape(E, 128, 512)
        .astype(np.float16)
    )  # wk[e, p, (ot*2+kt)*128 + oc] = W[e, kt*128+p, ot*128+oc]

    in_maps = []
    for e in range(E):
        t = toks[e]
        xs = np.zeros((128, 2, C), dtype=np.float16)
        xs[:, :, : len(t)] = xT[:, :, t].transpose(1, 0, 2)
        in_maps.append({"wq": np.ascontiguousarray(wk[e]), "xq": xs})
    return in_maps, (C, toks)


def kernel(x, W, Wp, bp):
    x = np.ascontiguousarray(np.asarray(x, dtype=np.float32))
    W = np.ascontiguousarray(np.asarray(W, dtype=np.float32))
    Wp = np.ascontiguousarray(np.asarray(Wp, dtype=np.float32))
    bp = np.ascontiguousarray(np.asarray(bp, dtype=np.float32))
    B, I = x.shape
    E, _, O = W.shape

    in_maps, (C, toks) = _pack_inputs(x, W, Wp, bp)
    nc = _get_program(I, O, C)
    res = run_bass_kernel_spmd(nc, in_maps, list(range(NCORES)))

    # host unscatter: y[token] += column (add handles tie multi-hot rows)
    y = np.zeros((B, O), dtype=np.float32)
    for e in range(E):
        t = toks[e]
        yc = res.results[e]["yq"]  # [2, 128, C] fp16
        ycols = yc.transpose(2, 0, 1).reshape(C, O)[: len(t)]
        np.add.at(y, t, ycols.astype(np.float32))
    return y
